# revision 1
# baseline (speedup 1.0000x reference)
"""Bass/Trainium2 kernel for nn_BridgeNodes: per-group thresholded sigmoid
similarity map  out[g] = where(sigmoid(nodes_g @ nodes_g.T) < 0.6, 0, sigmoid(...)).

The map is exactly symmetric (dot(i,j) and dot(j,i) accumulate in the same
order on the PE), so only upper-triangle tiles are computed on device; the
host mirrors the lower triangle during unshard.

Sharding: 8 cores = (group, row-parity). Core i handles group i//2 and the
16 row-blocks m = 2k + (i%2) (k=0..15, 128 rows each) of that group. For
row-block m only column chunks j >= floor(m/4) (512 cols each) are computed
— chunk counts per k are parity-independent, so one SPMD program serves all
cores; the host supplies each core's row-blocks gathered into rows_t.

Per-chunk pipeline:
  PE    : matmul [K=128, M=128, N=512] -> PSUM  (dot = x, native fp32)
  ACT   : s = Sigmoid(psum)            -> SBUF
  DVE   : out = (psum >= c) * s        -> SBUF   (one fused
          scalar_tensor_tensor: op0=is_ge vs c, op1=mult by s;
          mask decided on the raw fp32 dot, exact 0.0 for dropped)
  DMA   : one store per row-block of the computed column suffix
"""

import numpy as np

import concourse.bacc as bacc
import concourse.bass as bass
import concourse.mybir as mybir
import concourse.tile as tile
from concourse.bass_utils import run_bass_kernel_spmd

G = 4          # groups
N = 4096       # nodes per group
F = 128        # feature dim
CORES = 8
MT = 128       # rows per m-tile (PSUM partition dim)
NB = N // MT   # 32 row-blocks per group
KT = NB // 2   # 16 row-blocks per core
R = KT * MT    # 2048 rows handled per core
CW = 512       # columns per chunk (one PSUM bank of fp32)

# Decision boundary in dot space: smallest fp32 x with sigmoid(x) >= f32(0.6).
# fp64-exact boundary is f32(ln 1.5) + 4 ulp = 0x3ecf9923.
THRESH_C = float(np.frombuffer(np.uint32(0x3ECF9923).tobytes(), np.float32)[0])


def _c0(k):
    # first computed column for local row-block k: the diagonal of global
    # row-block m = 2k+p starts at m*128; 2k*128 = k*256 covers both
    # parities (p=1 recomputes 128 sub-diagonal cols, overwritten by the
    # host mirror)
    return k * 2 * MT


def _w(k):
    # computed width (cols) for local row-block k
    return N - _c0(k)


_OFF = np.concatenate([[0], np.cumsum([_w(k) for k in range(KT)])]).astype(int)
TOTW = int(_OFF[-1])  # 34816 — packed output cols

_NC_CACHE = {}


def _j0(k):
    # first computed 512-col chunk for local row-block k (global m = 2k+p;
    # floor((2k+p)/4) is parity-independent)
    return (2 * k) // 4


def _build_nc():
    if "nc" in _NC_CACHE:
        return _NC_CACHE["nc"]
    f32 = mybir.dt.float32
    nc = bacc.Bacc()
    rows_t = nc.dram_tensor("rows_t", [F, R], f32, kind="ExternalInput")
    cols_t = nc.dram_tensor("cols_t", [F, N], f32, kind="ExternalInput")
    out = nc.dram_tensor("out", [MT, TOTW], f32, kind="ExternalOutput")

    with tile.TileContext(nc) as tc:
        with (
            tc.tile_pool(name="inp", bufs=1) as inp,
            tc.tile_pool(name="ps", bufs=8, space="PSUM") as psp,
            tc.tile_pool(name="sig", bufs=3) as sigp,
            tc.tile_pool(name="res", bufs=3) as resp,
        ):
            rt = inp.tile([F, R], f32)
            ct = inp.tile([F, N], f32)
            # split loads so the first matmuls start as soon as their
            # slices land instead of waiting for the full 3 MiB; each
            # dma_start costs ~0.6us of serial HWDGE dispatch, so keep
            # the piece count low
            nc.sync.dma_start(ct[:, :CW], cols_t[:, :CW])
            nc.sync.dma_start(rt[:, :MT], rows_t[:, :MT])
            nc.sync.dma_start(ct[:, CW : 3 * CW], cols_t[:, CW : 3 * CW])
            nc.sync.dma_start(ct[:, 3 * CW : 5 * CW], cols_t[:, 3 * CW : 5 * CW])
            nc.sync.dma_start(ct[:, 5 * CW :], cols_t[:, 5 * CW :])
            nc.sync.dma_start(rt[:, MT:], rows_t[:, MT:])

            # prime the PE's activity monitor while inputs stream in: tiny
            # matmuls on a memset tile (no DMA dependency) keep the clock
            # gate ramping so the first real matmuls run warm
            wsrc = inp.tile([MT, 64], f32)
            nc.vector.memset(wsrc[:], 0.0)
            warm = psp.tile([MT, CW], f32, tag="ps")
            for _ in range(8):
                nc.tensor.matmul(warm[:64, :64], wsrc[:, :64], wsrc[:, :64])

            for k in range(KT):
                ncols = _w(k)
                s = sigp.tile([MT, ncols], f32, tag="sig")
                o = resp.tile([MT, ncols], f32, tag="res")
                for c in range(0, ncols, CW):
                    cw = min(CW, ncols - c)
                    col = _c0(k) + c
                    ps = psp.tile([MT, CW], f32)
                    nc.tensor.matmul(
                        ps[:, :cw],
                        rt[:, k * MT : (k + 1) * MT],
                        ct[:, col : col + cw],
                    )
                    sq = s[:, c : c + cw]
                    nc.scalar.activation(
                        sq, ps[:, :cw], mybir.ActivationFunctionType.Sigmoid
                    )
                    nc.vector.scalar_tensor_tensor(
                        o[:, c : c + cw],
                        ps[:, :cw],
                        THRESH_C,
                        sq,
                        op0=mybir.AluOpType.is_ge,
                        op1=mybir.AluOpType.mult,
                    )
                nc.sync.dma_start(out[:, _OFF[k] : _OFF[k + 1]], o[:])
    nc.finalize()
    _NC_CACHE["nc"] = nc
    return nc


def _in_maps(nodes):
    maps = []
    for core in range(CORES):
        g, p = core // 2, core % 2
        ct = np.ascontiguousarray(nodes[g].T)  # [F, N]
        # gather this core's row-blocks: m = 2k+p
        rt = np.ascontiguousarray(
            ct.reshape(F, NB, MT)[:, p::2, :].reshape(F, R)
        )
        maps.append({"rows_t": rt, "cols_t": ct})
    return maps


def _assemble(results):
    full = np.zeros((G, N, N), np.float32)
    for core in range(CORES):
        g, p = core // 2, core % 2
        packed = results[core]["out"]
        for k in range(KT):
            m = 2 * k + p
            full[g, m * MT : (m + 1) * MT, _c0(k):] = packed[:, _OFF[k] : _OFF[k + 1]]
    # mirror strictly-lower row-blocks from the computed upper triangle
    for g in range(G):
        x = full[g]
        for bi in range(NB):
            for bj in range(bi):
                x[bi * MT : (bi + 1) * MT, bj * MT : (bj + 1) * MT] = x[
                    bj * MT : (bj + 1) * MT, bi * MT : (bi + 1) * MT
                ].T
    return full


def kernel(nodes):
    nodes = np.ascontiguousarray(np.asarray(nodes, dtype=np.float32))
    assert nodes.shape == (G, N, F), nodes.shape
    nc = _build_nc()
    res = run_bass_kernel_spmd(nc, _in_maps(nodes), list(range(CORES))).results
    return _assemble(res)



# revision 3
# speedup vs baseline: 1.6661x; 1.6661x over previous
"""Bass/Trainium2 kernel for nn_BridgeNodes: per-group thresholded sigmoid
similarity map  out[g] = where(sigmoid(nodes_g @ nodes_g.T) < 0.6, 0, sigmoid(...)).

v2 design (3.5x over the v1 fp32 kernel):
  PE   : bf16 matmuls (1 cyc/row vs 4 for fp32) of the upper-triangle tiles.
  ACT  : y = Identity(SCALE*x - SCALE*c) -> fp8 e5m2   (~55% of columns)
  DVE  : y = (x - c) * SCALE             -> fp8 e5m2   (~45% of columns)
         Signed encoding: sign(y) carries the exact-on-device mask decision,
         |y|/SCALE is the distance from the threshold in dot space.
  DMA  : 1-byte/elem output (4.46 MB/core vs 17.8 MB in v1), bf16 inputs.

Host decode is a 256-entry LUT (pattern -> sigmoid(c + v/SCALE) if v>0 else 0).
bf16 inputs + e5m2 quantization blur the threshold decision inside a narrow
guard band |x - c| <= BAND (~4e-3, ~1e-3 of entries); the host recomputes
those few dots exactly from the original fp32 nodes and reapplies the exact
threshold. The map is symmetric, so only upper-triangle tiles are computed on
device; the host mirrors the lower triangle during unshard.

Sharding: 8 cores = (group, row-parity). Core i handles group i//2 and the
16 row-blocks m = 2k + (i%2) (k=0..15, 128 rows each) of that group. For
row-block m only columns >= 256*k are computed (parity-independent, so one
SPMD program serves all cores); sub-diagonal spill is overwritten by the
host mirror.
"""

import numpy as np
import ml_dtypes

import concourse.bacc as bacc
import concourse.bass as bass
import concourse.mybir as mybir
import concourse.tile as tile
from concourse.bass_utils import run_bass_kernel_spmd

G = 4          # groups
N = 4096       # nodes per group
F = 128        # feature dim
CORES = 8
MT = 128       # rows per m-tile (PSUM partition dim)
NB = N // MT   # 32 row-blocks per group
KT = NB // 2   # 16 row-blocks per core
R = KT * MT    # 2048 rows handled per core
EW = 2048      # elementwise chunk = 4 PSUM banks of fp32
MW = 512       # matmul sub-chunk = 1 PSUM bank

# Decision boundary in dot space: smallest fp32 x with sigmoid(x) >= f32(0.6).
THRESH_C = float(np.frombuffer(np.uint32(0x3ECF9923).tobytes(), np.float32)[0])
SCALE = 4096.0
BAND = 4e-3    # host-recompute guard band in dot space (~10 sigma of bf16 noise)


def _c0(k):
    # first computed column for local row-block k (global m = 2k+p; the host
    # mirror overwrites the sub-diagonal spill for p=1)
    return k * 2 * MT


def _w(k):
    return N - _c0(k)


_OFF = np.concatenate([[0], np.cumsum([_w(k) for k in range(KT)])]).astype(int)
TOTW = int(_OFF[-1])  # 34816 packed output cols per core

_NC_CACHE = {}


def _chunk_engines():
    """Greedy ACT/DVE balance over the k-major chunk sequence."""
    sched = {}
    act_t = dve_t = 0.0
    for k in range(KT):
        w = _w(k)
        for c in range(0, w, EW):
            cw = min(EW, w - c)
            a_cost = cw * 0.833 + 185.0
            d_cost = cw * 1.0417 + 130.0
            if act_t + a_cost <= dve_t + d_cost:
                sched[(k, c)] = "act"
                act_t += a_cost
            else:
                sched[(k, c)] = "dve"
                dve_t += d_cost
    return sched


def _build_nc():
    if "nc" in _NC_CACHE:
        return _NC_CACHE["nc"]
    f32 = mybir.dt.float32
    bf16 = mybir.dt.bfloat16
    f8 = mybir.dt.float8e5
    sched = _chunk_engines()

    nc = bacc.Bacc()
    rows_t = nc.dram_tensor("rows_t", [F, R], bf16, kind="ExternalInput")
    cols_t = nc.dram_tensor("cols_t", [F, N], bf16, kind="ExternalInput")
    out = nc.dram_tensor("out", [MT, TOTW], f8, kind="ExternalOutput")

    with tile.TileContext(nc) as tc:
        with (
            tc.tile_pool(name="inp", bufs=1) as inp,
            tc.tile_pool(name="ps", bufs=2, space="PSUM") as psp,
            tc.tile_pool(name="res", bufs=3) as resp,
        ):
            rt = inp.tile([F, R], bf16)
            ct = inp.tile([F, N], bf16)
            # split loads so the first matmuls start as soon as their slices
            # land; keep the piece count low (each dma_start costs ~0.6us of
            # serial HWDGE dispatch)
            nc.sync.dma_start(ct[:, :MW], cols_t[:, :MW])
            nc.sync.dma_start(rt[:, :MT], rows_t[:, :MT])
            nc.sync.dma_start(ct[:, MW : 4 * MW], cols_t[:, MW : 4 * MW])
            nc.sync.dma_start(ct[:, 4 * MW :], cols_t[:, 4 * MW :])
            nc.sync.dma_start(rt[:, MT:], rows_t[:, MT:])

            biast = inp.tile([MT, 1], f32)
            nc.vector.memset(biast[:], -SCALE * THRESH_C)

            # prime the PE clock gate while inputs stream in
            wsrc = inp.tile([MT, 64], bf16)
            nc.vector.memset(wsrc[:], 0.0)
            warm = psp.tile([MT, EW], f32, tag="ps")
            for _ in range(8):
                nc.tensor.matmul(warm[:64, :64], wsrc[:, :64], wsrc[:, :64])

            for k in range(KT):
                ncols = _w(k)
                o = resp.tile([MT, ncols], f8, tag="res")
                for c in range(0, ncols, EW):
                    cw = min(EW, ncols - c)
                    col = _c0(k) + c
                    ps = psp.tile([MT, EW], f32, tag="ps")
                    for mo in range(0, cw, MW):
                        mw = min(MW, cw - mo)
                        nc.tensor.matmul(
                            ps[:, mo : mo + mw],
                            rt[:, k * MT : (k + 1) * MT],
                            ct[:, col + mo : col + mo + mw],
                        )
                    oq = o[:, c : c + cw]
                    if sched[(k, c)] == "act":
                        # e5m2(SCALE*x - SCALE*c), sign = mask
                        nc.scalar.activation(
                            oq,
                            ps[:, :cw],
                            mybir.ActivationFunctionType.Identity,
                            bias=biast[:],
                            scale=SCALE,
                        )
                    else:
                        # e5m2((x - c) * SCALE), same encoding on DVE
                        nc.vector.tensor_scalar(
                            oq,
                            ps[:, :cw],
                            THRESH_C,
                            SCALE,
                            op0=mybir.AluOpType.subtract,
                            op1=mybir.AluOpType.mult,
                        )
                nc.sync.dma_start(out[:, _OFF[k] : _OFF[k + 1]], o[:])
    nc.finalize()
    _NC_CACHE["nc"] = nc
    return nc


def _in_maps(nodes):
    maps = []
    for core in range(CORES):
        g, p = core // 2, core % 2
        ct = np.ascontiguousarray(nodes[g].T).astype(ml_dtypes.bfloat16)  # [F, N]
        rt = np.ascontiguousarray(
            ct.reshape(F, NB, MT)[:, p::2, :].reshape(F, R)
        )
        maps.append({"rows_t": rt, "cols_t": ct})
    return maps


_LUT_CACHE = {}


def _luts():
    if "v" in _LUT_CACHE:
        return _LUT_CACHE["v"], _LUT_CACHE["f"]
    pats = (
        np.arange(256, dtype=np.uint8).view(ml_dtypes.float8_e5m2).astype(np.float64)
    )
    with np.errstate(over="ignore", invalid="ignore"):
        val = np.where(
            pats > 0, 1.0 / (1.0 + np.exp(-(THRESH_C + pats / SCALE))), 0.0
        ).astype(np.float32)
    val[~np.isfinite(pats)] = 0.0
    fix = (np.abs(pats) <= BAND * SCALE) | (pats == 0)
    fix[~np.isfinite(pats)] = False
    _LUT_CACHE["v"], _LUT_CACHE["f"] = val, fix
    return val, fix


def _assemble(results, nodes):
    val_lut, fix_lut = _luts()
    full = np.zeros((G, N, N), np.float32)
    fix_g, fix_i, fix_j = [], [], []
    for core in range(CORES):
        g, p = core // 2, core % 2
        packed = np.ascontiguousarray(results[core]["out"]).view(np.uint8)
        vals = val_lut[packed]  # [MT, TOTW] fp32
        for k in range(KT):
            m = 2 * k + p
            full[g, m * MT : (m + 1) * MT, _c0(k):] = vals[:, _OFF[k] : _OFF[k + 1]]
        rr, cc = np.nonzero(fix_lut[packed])
        if rr.size:
            kk = np.searchsorted(_OFF, cc, side="right") - 1
            fix_g.append(np.full(rr.shape, g))
            fix_i.append((2 * kk + p) * MT + rr)
            fix_j.append(cc - _OFF[kk] + kk * 2 * MT)
    # exact recompute of the guard-band entries from the original fp32 nodes
    if fix_g:
        gg = np.concatenate(fix_g)
        ii = np.concatenate(fix_i)
        jj = np.concatenate(fix_j)
        n64 = nodes.astype(np.float64)
        dots = np.einsum("kf,kf->k", n64[gg, ii], n64[gg, jj])
        sig = (1.0 / (1.0 + np.exp(-dots))).astype(np.float32)
        full[gg, ii, jj] = np.where(sig >= np.float32(0.6), sig, np.float32(0.0))
    # mirror strictly-lower row-blocks from the computed upper triangle
    for g in range(G):
        x = full[g]
        for bi in range(NB):
            for bj in range(bi):
                x[bi * MT : (bi + 1) * MT, bj * MT : (bj + 1) * MT] = x[
                    bj * MT : (bj + 1) * MT, bi * MT : (bi + 1) * MT
                ].T
    return full


def kernel(nodes):
    nodes = np.ascontiguousarray(np.asarray(nodes, dtype=np.float32))
    assert nodes.shape == (G, N, F), nodes.shape
    nc = _build_nc()
    res = run_bass_kernel_spmd(nc, _in_maps(nodes), list(range(CORES))).results
    return _assemble(res, nodes)


# revision 13
# speedup vs baseline: 2.2094x; 1.3260x over previous
"""Bass/Trainium2 kernel for nn_BridgeNodes: per-group thresholded sigmoid
similarity map  out[g] = where(sigmoid(nodes_g @ nodes_g.T) < 0.6, 0, sigmoid(...)).

v2 design (3.5x over the v1 fp32 kernel):
  PE   : bf16 matmuls (1 cyc/row vs 4 for fp32) of the upper-triangle tiles.
  ACT  : y = Identity(SCALE*x - SCALE*c) -> fp8 e5m2   (~55% of columns)
  DVE  : y = (x - c) * SCALE             -> fp8 e5m2   (~45% of columns)
         Signed encoding: sign(y) carries the exact-on-device mask decision,
         |y|/SCALE is the distance from the threshold in dot space.
  DMA  : 1-byte/elem output (4.46 MB/core vs 17.8 MB in v1), bf16 inputs.

Host decode is a 256-entry LUT (pattern -> sigmoid(c + v/SCALE) if v>0 else 0).
bf16 inputs + e5m2 quantization blur the threshold decision inside a narrow
guard band |x - c| <= BAND (~4e-3, ~1e-3 of entries); the host recomputes
those few dots exactly from the original fp32 nodes and reapplies the exact
threshold. The map is symmetric, so only upper-triangle tiles are computed on
device; the host mirrors the lower triangle during unshard.

Sharding: 8 cores = (group, row-parity). Core i handles group i//2 and the
16 row-blocks m = 2k + (i%2) (k=0..15, 128 rows each) of that group. For
row-block m only columns >= 256*k are computed (parity-independent, so one
SPMD program serves all cores); sub-diagonal spill is overwritten by the
host mirror.
"""

import numpy as np
import ml_dtypes

import concourse.bacc as bacc
import concourse.bass as bass
import concourse.mybir as mybir
import concourse.tile as tile
from concourse.bass_utils import run_bass_kernel_spmd

G = 4          # groups
N = 4096       # nodes per group
F = 128        # feature dim
CORES = 8
MT = 128       # rows per m-tile (PSUM partition dim)
NB = N // MT   # 32 row-blocks per group
KT = NB // 2   # 16 row-blocks per core
R = KT * MT    # 2048 rows handled per core
EW = 1024      # elementwise chunk = 2 PSUM banks of fp32
MW = 512       # matmul sub-chunk = 1 PSUM bank
PSUM_BUFS = 4  # psum tiles in flight (EW * PSUM_BUFS * 4B <= 16 KB/partition)
POOL_DMA = False  # issue every other output store from the Pool queue (SWDGE)

# Decision boundary in dot space: smallest fp32 x with sigmoid(x) >= f32(0.6).
THRESH_C = float(np.frombuffer(np.uint32(0x3ECF9923).tobytes(), np.float32)[0])
SCALE = 4096.0
BAND = 4e-3    # host-recompute guard band in dot space (~10 sigma of bf16 noise)


def _c0(k):
    # first computed column for local row-block k (global m = 2k+p; the host
    # mirror overwrites the sub-diagonal spill for p=1)
    return k * 2 * MT


def _w(k):
    return N - _c0(k)


_OFF = np.concatenate([[0], np.cumsum([_w(k) for k in range(KT)])]).astype(int)
TOTW = int(_OFF[-1])  # 34816 packed output cols per core

# row-block groups sharing one output tile + one store
GROUPS = [[0], [1], [2], [3], [4], [5], [6], [7], [8, 9], [10, 11, 12], [13, 14, 15]]

_NC_CACHE = {}


def _chunk_engines():
    """Greedy ACT/DVE balance over the k-major chunk sequence."""
    sched = {}
    act_t, dve_t = 1283.0, 0.0  # ACT pays a one-time table load
    for k in range(KT):
        w = _w(k)
        for c in range(0, w, EW):
            cw = min(EW, w - c)
            a_cost = cw * 0.833 + 185.0
            d_cost = cw * 1.0417 + 130.0
            if act_t + a_cost <= dve_t + d_cost:
                sched[(k, c)] = "act"
                act_t += a_cost
            else:
                sched[(k, c)] = "dve"
                dve_t += d_cost
    return sched


def _build_nc():
    if "nc" in _NC_CACHE:
        return _NC_CACHE["nc"]
    f32 = mybir.dt.float32
    bf16 = mybir.dt.bfloat16
    f8 = mybir.dt.float8e5
    sched = _chunk_engines()

    nc = bacc.Bacc()
    rows_t = nc.dram_tensor("rows_t", [F, R], bf16, kind="ExternalInput")
    cols_t = nc.dram_tensor("cols_t", [F, N], bf16, kind="ExternalInput")
    out = nc.dram_tensor("out", [MT, TOTW], f8, kind="ExternalOutput")

    with tile.TileContext(nc) as tc:
        with (
            tc.tile_pool(name="inp", bufs=1) as inp,
            tc.tile_pool(name="ps", bufs=PSUM_BUFS, space="PSUM") as psp,
            tc.tile_pool(name="res", bufs=4) as resp,
        ):
            rt = inp.tile([F, R], bf16)
            ct = inp.tile([F, N], bf16)
            # Parallel-queue input dispatch: every engine sequencer is idle at
            # t=0, and each dma_start costs ~0.6us of serial dispatch on its
            # issuing queue, so spread the pieces to get all transfers queued
            # on the DMA bus within ~1us. Pieces ordered by first consumption.
            nc.sync.dma_start(ct[:, :MW], cols_t[:, :MW])
            nc.scalar.dma_start(rt[:, :MT], rows_t[:, :MT])
            nc.gpsimd.dma_start(ct[:, MW : 3 * MW], cols_t[:, MW : 3 * MW])
            nc.sync.dma_start(ct[:, 3 * MW : 5 * MW], cols_t[:, 3 * MW : 5 * MW])
            nc.scalar.dma_start(ct[:, 5 * MW :], cols_t[:, 5 * MW :])
            nc.gpsimd.dma_start(rt[:, MT:], rows_t[:, MT:])

            biast = inp.tile([MT, 1], f32)
            nc.gpsimd.memset(biast[:], -SCALE * THRESH_C)

            # prime the PE clock gate while inputs stream in
            wsrc = inp.tile([MT, 64], bf16)
            nc.gpsimd.memset(wsrc[:], 0.0)
            warm = psp.tile([MT, EW], f32, tag="ps")
            for _ in range(8):
                nc.tensor.matmul(warm[:64, :64], wsrc[:, :64], wsrc[:, :64])
            # trigger the ACT table load during the input DMA, not on the
            # first real drain chunk
            prime = inp.tile([MT, 1], f8)
            nc.scalar.activation(
                prime[:],
                biast[:],
                mybir.ActivationFunctionType.Identity,
                bias=biast[:],
                scale=SCALE,
            )

            # group small row-blocks into shared output tiles / single stores
            # to cut store+semaphore churn at the tail
            for kgroup in GROUPS:
                k0 = kgroup[0]
                gw = int(_OFF[kgroup[-1] + 1] - _OFF[k0])
                o = resp.tile([MT, gw], f8, tag="res")
                for k in kgroup:
                    ncols = _w(k)
                    ob = int(_OFF[k] - _OFF[k0])
                    for c in range(0, ncols, EW):
                        cw = min(EW, ncols - c)
                        col = _c0(k) + c
                        ps = psp.tile([MT, EW], f32, tag="ps")
                        for mo in range(0, cw, MW):
                            mw = min(MW, cw - mo)
                            nc.tensor.matmul(
                                ps[:, mo : mo + mw],
                                rt[:, k * MT : (k + 1) * MT],
                                ct[:, col + mo : col + mo + mw],
                            )
                        oq = o[:, ob + c : ob + c + cw]
                        if sched[(k, c)] == "act":
                            # e5m2(SCALE*x - SCALE*c), sign = mask
                            nc.scalar.activation(
                                oq,
                                ps[:, :cw],
                                mybir.ActivationFunctionType.Identity,
                                bias=biast[:],
                                scale=SCALE,
                            )
                        else:
                            # e5m2((x - c) * SCALE), same encoding on DVE
                            nc.vector.tensor_scalar(
                                oq,
                                ps[:, :cw],
                                THRESH_C,
                                SCALE,
                                op0=mybir.AluOpType.subtract,
                                op1=mybir.AluOpType.mult,
                            )
                nc.sync.dma_start(
                    out[:, _OFF[k0] : _OFF[kgroup[-1] + 1]], o[:]
                )
    nc.finalize()
    _NC_CACHE["nc"] = nc
    return nc


def _in_maps(nodes):
    maps = []
    for core in range(CORES):
        g, p = core // 2, core % 2
        ct = np.ascontiguousarray(nodes[g].T).astype(ml_dtypes.bfloat16)  # [F, N]
        rt = np.ascontiguousarray(
            ct.reshape(F, NB, MT)[:, p::2, :].reshape(F, R)
        )
        maps.append({"rows_t": rt, "cols_t": ct})
    return maps


_LUT_CACHE = {}


def _luts():
    if "v" in _LUT_CACHE:
        return _LUT_CACHE["v"], _LUT_CACHE["f"]
    pats = (
        np.arange(256, dtype=np.uint8).view(ml_dtypes.float8_e5m2).astype(np.float64)
    )
    with np.errstate(over="ignore", invalid="ignore"):
        val = np.where(
            pats > 0, 1.0 / (1.0 + np.exp(-(THRESH_C + pats / SCALE))), 0.0
        ).astype(np.float32)
    val[~np.isfinite(pats)] = 0.0
    fix = (np.abs(pats) <= BAND * SCALE) | (pats == 0)
    fix[~np.isfinite(pats)] = False
    _LUT_CACHE["v"], _LUT_CACHE["f"] = val, fix
    return val, fix


def _assemble(results, nodes):
    val_lut, fix_lut = _luts()
    full = np.zeros((G, N, N), np.float32)
    fix_g, fix_i, fix_j = [], [], []
    for core in range(CORES):
        g, p = core // 2, core % 2
        packed = np.ascontiguousarray(results[core]["out"]).view(np.uint8)
        vals = val_lut[packed]  # [MT, TOTW] fp32
        for k in range(KT):
            m = 2 * k + p
            full[g, m * MT : (m + 1) * MT, _c0(k):] = vals[:, _OFF[k] : _OFF[k + 1]]
        rr, cc = np.nonzero(fix_lut[packed])
        if rr.size:
            kk = np.searchsorted(_OFF, cc, side="right") - 1
            fix_g.append(np.full(rr.shape, g))
            fix_i.append((2 * kk + p) * MT + rr)
            fix_j.append(cc - _OFF[kk] + kk * 2 * MT)
    # exact recompute of the guard-band entries from the original fp32 nodes
    if fix_g:
        gg = np.concatenate(fix_g)
        ii = np.concatenate(fix_i)
        jj = np.concatenate(fix_j)
        n64 = nodes.astype(np.float64)
        dots = np.einsum("kf,kf->k", n64[gg, ii], n64[gg, jj])
        sig = (1.0 / (1.0 + np.exp(-dots))).astype(np.float32)
        full[gg, ii, jj] = np.where(sig >= np.float32(0.6), sig, np.float32(0.0))
    # mirror strictly-lower row-blocks from the computed upper triangle
    for g in range(G):
        x = full[g]
        for bi in range(NB):
            for bj in range(bi):
                x[bi * MT : (bi + 1) * MT, bj * MT : (bj + 1) * MT] = x[
                    bj * MT : (bj + 1) * MT, bi * MT : (bi + 1) * MT
                ].T
    return full


def kernel(nodes):
    nodes = np.ascontiguousarray(np.asarray(nodes, dtype=np.float32))
    assert nodes.shape == (G, N, F), nodes.shape
    nc = _build_nc()
    res = run_bass_kernel_spmd(nc, _in_maps(nodes), list(range(CORES))).results
    return _assemble(res, nodes)


# revision 33
# speedup vs baseline: 2.4439x; 1.1061x over previous
"""Bass/Trainium2 kernel for nn_BridgeNodes: per-group thresholded sigmoid
similarity map  out[g] = where(sigmoid(nodes_g @ nodes_g.T) < 0.6, 0, sigmoid(...)).

v2 design (3.5x over the v1 fp32 kernel):
  PE   : bf16 matmuls (1 cyc/row vs 4 for fp32) of the upper-triangle tiles.
  ACT  : y = Identity(SCALE*x - SCALE*c) -> fp8 e5m2   (~55% of columns)
  DVE  : y = (x - c) * SCALE             -> fp8 e5m2   (~45% of columns)
         Signed encoding: sign(y) carries the exact-on-device mask decision,
         |y|/SCALE is the distance from the threshold in dot space.
  DMA  : 1-byte/elem output (4.46 MB/core vs 17.8 MB in v1), bf16 inputs.

Host decode is a 256-entry LUT (pattern -> sigmoid(c + v/SCALE) if v>0 else 0).
bf16 inputs + e5m2 quantization blur the threshold decision inside a narrow
guard band |x - c| <= BAND (~4e-3, ~1e-3 of entries); the host recomputes
those few dots exactly from the original fp32 nodes and reapplies the exact
threshold. The map is symmetric, so only upper-triangle tiles are computed on
device; the host mirrors the lower triangle during unshard.

Sharding: 8 cores = (group, row-parity). Core i handles group i//2 and the
16 row-blocks m = 2k + (i%2) (k=0..15, 128 rows each) of that group. For
row-block m only columns >= 256*k are computed (parity-independent, so one
SPMD program serves all cores); sub-diagonal spill is overwritten by the
host mirror.
"""

import numpy as np
import ml_dtypes

import concourse.bacc as bacc
import concourse.bass as bass
import concourse.mybir as mybir
import concourse.tile as tile
from concourse.bass_utils import run_bass_kernel_spmd

G = 4          # groups
N = 4096       # nodes per group
F = 128        # feature dim
CORES = 8
MT = 128       # rows per m-tile (PSUM partition dim)
NB = N // MT   # 32 row-blocks per group
KT = NB // 2   # 16 row-blocks per core
R = KT * MT    # 2048 rows handled per core
EW = 1024      # elementwise chunk = 2 PSUM banks of fp32
MW = 512       # matmul sub-chunk = 1 PSUM bank
PSUM_BUFS = 4  # psum tiles in flight (EW * PSUM_BUFS * 4B <= 16 KB/partition)
POOL_DMA = False  # issue every other output store from the Pool queue (SWDGE)

# Decision boundary in dot space: smallest fp32 x with sigmoid(x) >= f32(0.6).
THRESH_C = float(np.frombuffer(np.uint32(0x3ECF9923).tobytes(), np.float32)[0])
SCALE = 4096.0
BAND = 4e-3    # host-recompute guard band in dot space (~10 sigma of bf16 noise)


def _c0(k):
    # first computed column for local row-block k (global m = 2k+p; the host
    # mirror overwrites the sub-diagonal spill for p=1)
    return k * 2 * MT


def _w(k):
    return N - _c0(k)


_OFF = np.concatenate([[0], np.cumsum([_w(k) for k in range(KT)])]).astype(int)
TOTW = int(_OFF[-1])  # 34816 packed output cols per core

# Row-block groups sharing one output tile + one store, in PROCESS order.
# The tail half (k=15..8) runs first: those blocks only need the high input
# columns, so draining starts while the low columns are still loading, and
# the big dense blocks (k=0..7) run after all input has landed.
GROUPS = [
    [15, 14, 13, 12],
    [11, 10],
    [9, 8],
    [0], [1], [2], [3], [4], [5], [6], [7],
]

_NC_CACHE = {}


def _chunks_of(k):
    """Chunk (offset, width) list for row-block k."""
    w = _w(k)
    out = []
    c = 0
    while c < w:
        cw = min(EW, w - c)
        out.append((c, cw))
        c += cw
    return out


def _chunk_engines():
    """Greedy ACT/DVE balance over the chunk sequence in process order."""
    sched = {}
    act_t, dve_t = 1200.0, 0.0  # bias: ACT runs ~6% hotter than its model
    for kgroup in GROUPS:
        for k in kgroup:
            for c, cw in _chunks_of(k):
                a_cost = cw * 0.833 + 185.0
                d_cost = cw * 1.0417 + 130.0
                if act_t + a_cost <= dve_t + d_cost:
                    sched[(k, c)] = "act"
                    act_t += a_cost
                else:
                    sched[(k, c)] = "dve"
                    dve_t += d_cost
    return sched


def _build_nc():
    if "nc" in _NC_CACHE:
        return _NC_CACHE["nc"]
    f32 = mybir.dt.float32
    bf16 = mybir.dt.bfloat16
    f8 = mybir.dt.float8e5
    sched = _chunk_engines()

    nc = bacc.Bacc()
    rows_t = nc.dram_tensor("rows_t", [F, R], bf16, kind="ExternalInput")
    cols_t = nc.dram_tensor("cols_t", [F, N], bf16, kind="ExternalInput")
    out = nc.dram_tensor("out", [MT, TOTW], f8, kind="ExternalOutput")

    with tile.TileContext(nc) as tc:
        with (
            tc.tile_pool(name="inp", bufs=1) as inp,
            tc.tile_pool(name="ps", bufs=PSUM_BUFS, space="PSUM") as psp,
            tc.tile_pool(name="res", bufs=4) as resp,
        ):
            rt = inp.tile([F, R], bf16)
            ct = inp.tile([F, N], bf16)
            # Parallel-queue input dispatch: every engine sequencer is idle at
            # t=0, and each dma_start costs ~0.6us of serial dispatch on its
            # issuing queue, so spread the pieces to get all transfers queued
            # on the DMA bus within ~1us. Pieces ordered by first consumption.
            # Pool's SWDGE path reaches first-byte fastest, so it carries the
            # first-needed pieces; SP/Act alternate so HWDGE emits the rest
            # in consumption order.
            # measured arrival order of queue slots: SP1, POOL1, ACT1, SP2,
            # ACT2, SP3, POOL2 — map pieces so they land in consumption
            # order (tail rows + high columns first)
            nc.sync.dma_start(rt[:, R // 2 :], rows_t[:, R // 2 :])  # SP1: rows k=8..15
            nc.gpsimd.dma_start(ct[:, 7 * MW :], cols_t[:, 7 * MW :])  # POOL1: cols 3584..4096
            nc.scalar.dma_start(ct[:, 5 * MW : 7 * MW], cols_t[:, 5 * MW : 7 * MW])  # ACT1
            nc.sync.dma_start(ct[:, 4 * MW : 5 * MW], cols_t[:, 4 * MW : 5 * MW])    # SP2
            nc.scalar.dma_start(ct[:, : 2 * MW], cols_t[:, : 2 * MW])  # ACT2: cols 0..1024
            nc.sync.dma_start(ct[:, 2 * MW : 4 * MW], cols_t[:, 2 * MW : 4 * MW])    # SP3
            nc.gpsimd.dma_start(rt[:, : R // 2], rows_t[:, : R // 2])  # POOL2: rows k=0..7

            biast = inp.tile([MT, 1], f32)
            nc.vector.memset(biast[:], -SCALE * THRESH_C)

            # prime the PE clock gate while inputs stream in
            wsrc = inp.tile([MT, 64], bf16)
            nc.vector.memset(wsrc[:], 0.0)
            warm = psp.tile([MT, EW], f32, tag="ps")
            for _ in range(8):
                nc.tensor.matmul(warm[:64, :64], wsrc[:, :64], wsrc[:, :64])
            # trigger the ACT table load during the input DMA, not on the
            # first real drain chunk
            prime = inp.tile([MT, 1], f8)
            nc.scalar.activation(
                prime[:],
                biast[:],
                mybir.ActivationFunctionType.Identity,
                bias=biast[:],
                scale=SCALE,
            )

            # group small row-blocks into shared output tiles / single stores
            # to cut store+semaphore churn at the tail
            for kgroup in GROUPS:
                k0 = min(kgroup)
                k1 = max(kgroup)
                gw = int(_OFF[k1 + 1] - _OFF[k0])
                o = resp.tile([MT, gw], f8, tag="res")
                for k in kgroup:
                    ob = int(_OFF[k] - _OFF[k0])
                    for c, cw in _chunks_of(k):
                        col = _c0(k) + c
                        ps = psp.tile([MT, EW], f32, tag="ps")
                        for mo in range(0, cw, MW):
                            mw = min(MW, cw - mo)
                            nc.tensor.matmul(
                                ps[:, mo : mo + mw],
                                rt[:, k * MT : (k + 1) * MT],
                                ct[:, col + mo : col + mo + mw],
                            )
                        oq = o[:, ob + c : ob + c + cw]
                        if sched[(k, c)] == "act":
                            # e5m2(SCALE*x - SCALE*c), sign = mask
                            nc.scalar.activation(
                                oq,
                                ps[:, :cw],
                                mybir.ActivationFunctionType.Identity,
                                bias=biast[:],
                                scale=SCALE,
                            )
                        else:
                            # e5m2((x - c) * SCALE), same encoding on DVE
                            nc.vector.tensor_scalar(
                                oq,
                                ps[:, :cw],
                                THRESH_C,
                                SCALE,
                                op0=mybir.AluOpType.subtract,
                                op1=mybir.AluOpType.mult,
                            )
                # spread the final stores over independent queues so their
                # dispatch pipelines overlap at the tail
                tail = len(GROUPS) - 1 - GROUPS.index(kgroup)
                eng = (
                    nc.sync
                    if tail >= 3
                    else [nc.gpsimd, nc.scalar, nc.gpsimd][tail]
                )
                if tail == 0 and gw > 2 * MW:
                    # split the very last store: the bulk ships while the
                    # final chunks still drain; only a sliver trails
                    eng.dma_start(
                        out[:, _OFF[k0] : _OFF[k1 + 1] - MW], o[:, : gw - MW]
                    )
                    nc.scalar.dma_start(
                        out[:, _OFF[k1 + 1] - MW : _OFF[k1 + 1]], o[:, gw - MW :]
                    )
                else:
                    eng.dma_start(out[:, _OFF[k0] : _OFF[k1 + 1]], o[:])
    nc.finalize()
    _NC_CACHE["nc"] = nc
    return nc


def _in_maps(nodes):
    maps = []
    for core in range(CORES):
        g, p = core // 2, core % 2
        ct = np.ascontiguousarray(nodes[g].T).astype(ml_dtypes.bfloat16)  # [F, N]
        rt = np.ascontiguousarray(
            ct.reshape(F, NB, MT)[:, p::2, :].reshape(F, R)
        )
        maps.append({"rows_t": rt, "cols_t": ct})
    return maps


_LUT_CACHE = {}


def _luts():
    if "v" in _LUT_CACHE:
        return _LUT_CACHE["v"], _LUT_CACHE["f"]
    pats = (
        np.arange(256, dtype=np.uint8).view(ml_dtypes.float8_e5m2).astype(np.float64)
    )
    with np.errstate(over="ignore", invalid="ignore"):
        val = np.where(
            pats > 0, 1.0 / (1.0 + np.exp(-(THRESH_C + pats / SCALE))), 0.0
        ).astype(np.float32)
    val[~np.isfinite(pats)] = 0.0
    fix = (np.abs(pats) <= BAND * SCALE) | (pats == 0)
    fix[~np.isfinite(pats)] = False
    _LUT_CACHE["v"], _LUT_CACHE["f"] = val, fix
    return val, fix


def _assemble(results, nodes):
    val_lut, fix_lut = _luts()
    full = np.zeros((G, N, N), np.float32)
    fix_g, fix_i, fix_j = [], [], []
    for core in range(CORES):
        g, p = core // 2, core % 2
        packed = np.ascontiguousarray(results[core]["out"]).view(np.uint8)
        vals = val_lut[packed]  # [MT, TOTW] fp32
        for k in range(KT):
            m = 2 * k + p
            full[g, m * MT : (m + 1) * MT, _c0(k):] = vals[:, _OFF[k] : _OFF[k + 1]]
        rr, cc = np.nonzero(fix_lut[packed])
        if rr.size:
            kk = np.searchsorted(_OFF, cc, side="right") - 1
            fix_g.append(np.full(rr.shape, g))
            fix_i.append((2 * kk + p) * MT + rr)
            fix_j.append(cc - _OFF[kk] + kk * 2 * MT)
    # exact recompute of the guard-band entries from the original fp32 nodes
    if fix_g:
        gg = np.concatenate(fix_g)
        ii = np.concatenate(fix_i)
        jj = np.concatenate(fix_j)
        n64 = nodes.astype(np.float64)
        dots = np.einsum("kf,kf->k", n64[gg, ii], n64[gg, jj])
        sig = (1.0 / (1.0 + np.exp(-dots))).astype(np.float32)
        full[gg, ii, jj] = np.where(sig >= np.float32(0.6), sig, np.float32(0.0))
    # mirror strictly-lower row-blocks from the computed upper triangle
    for g in range(G):
        x = full[g]
        for bi in range(NB):
            for bj in range(bi):
                x[bi * MT : (bi + 1) * MT, bj * MT : (bj + 1) * MT] = x[
                    bj * MT : (bj + 1) * MT, bi * MT : (bi + 1) * MT
                ].T
    return full


def kernel(nodes):
    nodes = np.ascontiguousarray(np.asarray(nodes, dtype=np.float32))
    assert nodes.shape == (G, N, F), nodes.shape
    nc = _build_nc()
    res = run_bass_kernel_spmd(nc, _in_maps(nodes), list(range(CORES))).results
    return _assemble(res, nodes)
